# revision 48
# baseline (speedup 1.0000x reference)
"""DNFNet localization kernel for Trainium2 (8 NeuronCores, data-parallel).

Computes, for x (2048, 256), mu (1024, 256), sigma (1, 1024, 256), temperature ():
    dist[b, f]  = sqrt(sum_d (sigma[f, d] * (x[b, d] - mu[f, d]))^2)
    loc         = exp(-dist)
    out         = softmax(sigmoid(temperature) * loc, axis=-1)

Strategy: expand the weighted squared distance into matmuls,
    dist2 = (x^2) @ s2^T  -  2 x @ (s2*mu)^T  +  c,     s2 = sigma^2,
    c[f]  = sum_d s2[f, d] * mu[f, d]^2,
so the O(B*F*D) work runs on the TensorEngine (float32r, 1 cyc/row).
The batch axis is sharded 8 ways; mu/sigma are replicated per core.

Per-core pipeline (B_c = 256 batch rows, 2 m-tiles, 2 n-chunks of 512):
  1. DMA sigma/mu in halves on the sync HWDGE queue; x/temperature on the
     scalar queue in parallel. A short stream of junk matmuls warms the
     PE clock gate during the wait.
  2. PE-transpose sigma/mu 128x128 chunks into d-major layout, fusing the
     PSUM->SBUF copy into the elementwise prep:
        W1T = Square(sigmaT)   (ACT, PSUM->SBUF)
        W2T = W1T * muT        (DVE)
        w3  = W2T * muT        (DVE, one (128,512) chunk per (jg, kd))
  3. PE-transpose x; xsqT = Square(xT) (ACT), xm2T = -2*xT (DVE).
  4. 6-matmul float32r PSUM accumulation per (m-tile, n-chunk):
        xsqT_kd @ W1T_kd + xm2T_kd @ W2T_kd + ones @ w3_kd
     (the ones-lhsT k-tiles add the batch-independent c term).
  5. Chunked ACT epilogue the moment each chain lands, all in the single
     natural_log_exp table set (sqrt(u) = exp(0.5 ln u); sigmoid via exp;
     no table reload thrash):
        ln -> exp(0.5 .) -> exp(-.) per chunk, then a 1024-wide
        exp(g*loc) with fused row-sum (accum_out), DVE reciprocal +
        scale, chunked DMA out.
"""

import os

import numpy as np

B = 2048
D = 256
F = 1024
NCORES = 8
BC = B // NCORES  # 256 batch rows per core
MT = BC // 128  # 2 m-tiles
KD = D // 128  # 2 k-tiles over the feature dim
FJ = F // 128  # 8 formula tiles of 128
JG = FJ // 4  # 2 groups of 4 formula tiles (512-wide n-chunks)

def build_bass(use_f32r=True):
    import concourse.bass as bass
    import concourse.mybir as mybir
    import concourse.tile as tile
    from concourse import bacc
    from concourse.bass import ds
    from concourse.masks import make_identity

    f32 = mybir.dt.float32
    fr = mybir.dt.float32r if use_f32r else f32
    AF = mybir.ActivationFunctionType

    class _Bacc(bacc.Bacc):
        """Bacc whose ACT-table chooser is steered to the one set that
        contains every function this kernel uses (Exp, Ln, Square), so the
        whole kernel needs a single table load instead of thrashing between
        the exp-only and ln-only sets. The set contents are real; only the
        greedy chooser's view of the other sets is narrowed."""

        def insert_act_table_loads(self):
            import bass_rust as _bass_rust

            from concourse.hw_specs import get_activation_tables

            has_activation = any(
                isinstance(i, mybir.InstActivation)
                for b in self.main_func.blocks
                for i in b.instructions
            )
            if not has_activation:
                return
            want = {AF.Exp, AF.Ln, AF.Square}
            tables = []
            for name, funcs in get_activation_tables(self.m.arch).items():
                if name != "natural_log_exp_and_others":
                    funcs = funcs - want
                tables.append((name, funcs))
            _bass_rust.insert_act_table_loads(self, tables)

    nc = _Bacc(trn_type="TRN2", target_bir_lowering=False, debug=False)

    x_d = nc.dram_tensor("x", [BC, D], f32, kind="ExternalInput").ap()
    mu_d = nc.dram_tensor("mu", [F, D], f32, kind="ExternalInput").ap()
    sig_d = nc.dram_tensor("sigma", [F, D], f32, kind="ExternalInput").ap()
    tmp_d = nc.dram_tensor("temp", [1, 1], f32, kind="ExternalInput").ap()
    out_d = nc.dram_tensor("out", [BC, F], f32, kind="ExternalOutput").ap()

    with tile.TileContext(nc) as tc:
        with (
            tc.tile_pool(name="const", bufs=1) as constp,
            tc.tile_pool(name="raw", bufs=1) as rawp,
            tc.tile_pool(name="wmats", bufs=1) as wp,
            tc.tile_pool(name="w3t", bufs=3) as w3p,
            tc.tile_pool(name="lhs", bufs=1) as lhsp,
            tc.tile_pool(name="epi", bufs=2) as epip,
            tc.tile_pool(name="small", bufs=2) as smallp,
            tc.tile_pool(name="tp", bufs=4, space="PSUM") as tpp,
            tc.tile_pool(name="ops", bufs=2, space="PSUM") as opsp,
        ):
            # ---- constants ----
            ident = constp.tile([128, 128], f32, tag="ident")
            make_identity(nc, ident[:, :])
            ones_f = constp.tile([128, 128], f32, tag="onesf")
            nc.gpsimd.memset(ones_f[:, :], 1.0)
            ones_t = constp.tile([128, 128], fr, tag="ones")
            nc.vector.tensor_copy(ones_t[:, :], ones_f[:, :])

            # ---- raw input loads ----
            # sigma/mu stream on the sync HWDGE queue in two 4-f-tile
            # groups (512-wide n-chunks minimize ACT per-op overhead; finer
            # splits trade start latency 1:1 against extra overhead since
            # the ACT engine stays saturated either way). x/temp ride the
            # scalar HWDGE queue concurrently.
            GROUPS = [(0, 4), (4, 4)]  # (first f-tile j0, n f-tiles)
            sig_all = rawp.tile([128, FJ * D], f32, tag="sig")  # (p, (j d))
            mu_all = rawp.tile([128, FJ * D], f32, tag="mu")
            sig_r = sig_d.rearrange("(j p) d -> p j d", p=128)
            mu_r = mu_d.rearrange("(j p) d -> p j d", p=128)
            for j0, nj in GROUPS:
                grp = ds(j0 * D, nj * D)
                nc.sync.dma_start(sig_all[:, grp], sig_r[:, j0 : j0 + nj, :])
                nc.sync.dma_start(mu_all[:, grp], mu_r[:, j0 : j0 + nj, :])
            x_all = rawp.tile([128, MT * D], f32, tag="x")  # (p, (m d))
            nc.scalar.dma_start(
                x_all[:, :], x_d.rearrange("(m p) d -> p m d", p=128)
            )
            t_col = constp.tile([128, 1], f32, tag="tcol")
            nc.scalar.dma_start(t_col[:, :], tmp_d.partition_broadcast(128))

            # ---- g = sigmoid(temperature) on all partitions ----
            # computed as 1/(1+exp(-t)) so the only ACT tables the kernel
            # ever needs are the natural_log_exp set (Ln/Exp/Square/Copy):
            # a single table load, no reload thrash.
            u_col = constp.tile([128, 1], f32, tag="ucol")
            nc.scalar.activation(u_col[:, :], t_col[:, :], AF.Exp, scale=-1.0)
            u1_col = constp.tile([128, 1], f32, tag="u1col")
            nc.vector.tensor_scalar_add(u1_col[:, :], u_col[:, :], 1.0)
            g_col = constp.tile([128, 1], f32, tag="gcol")
            nc.vector.reciprocal(g_col[:, :], u1_col[:, :])

            # ---- PE warmup during the input DMA wait ----
            # ~7 junk fp32 matmuls (ones x ones) keep the PE busy from t~0.4
            # so the HAM clock gate reaches full speed before the real
            # transposes arrive.
            warm_ps = opsp.tile([128, 128], f32, tag="ops", name="warm_ps")
            for _ in range(7):
                nc.tensor.matmul(
                    warm_ps[:, 0:128],
                    ones_f[:, :],
                    ones_f[:, :],
                    start=True,
                    stop=True,
                )

            # ---- x transposes -> xsqT, xm2T (d-major lhsT tiles) ----
            xsqT = []
            xm2T = []
            for kd in range(KD):
                xtp = tpp.tile([128, 512], f32, tag="tp")
                for mi in range(MT):
                    nc.tensor.transpose(
                        xtp[:, ds(mi * 128, 128)],
                        x_all[:, ds(mi * D + kd * 128, 128)],
                        ident[:, :],
                    )
                xsq = lhsp.tile([128, MT * 128], fr, tag=f"xsq{kd}", name=f"xsq{kd}")
                nc.scalar.square(xsq[:, :], xtp[:, 0 : MT * 128])
                xm2 = lhsp.tile([128, MT * 128], fr, tag=f"xm2{kd}", name=f"xm2{kd}")
                nc.vector.tensor_scalar_mul(xm2[:, :], xtp[:, 0 : MT * 128], -2.0)
                xsqT.append(xsq)
                xm2T.append(xm2)

            # ---- W matrices in d-major layout ----
            w1t = [wp.tile([128, F], fr, tag=f"w1t{kd}", name=f"w1t{kd}") for kd in range(KD)]
            w2t = [wp.tile([128, F], fr, tag=f"w2t{kd}", name=f"w2t{kd}") for kd in range(KD)]
            # w3[jg][kd]: (sigma^2 mu^2)^T chunks, applied as two ones-lhsT
            # k-tiles per chain (no pre-add: keeps them off the chain's
            # critical path)
            w3c = {}

            # ---- W prep for all n-chunks, then chains + chunk epilogue ----
            # sqrt(d2) = exp(0.5*ln(d2)) keeps a single ACT table set.
            ops_mi = [
                opsp.tile([128, F], f32, tag="ops", name=f"ops{mi}")
                for mi in range(MT)
            ]
            loc_mi = [
                epip.tile([128, F], f32, tag=f"loc{mi}", name=f"loc{mi}", bufs=1)
                for mi in range(MT)
            ]
            for gi, (j0, nj) in enumerate(GROUPS):
                jgs = ds(j0 * 128, nj * 128)
                # both kd's W1/W2 first (they gate the data matmuls of the
                # chain), then the w3 products (they only gate the final two
                # c k-tiles)
                # sigma arrives ~1.6us before mu: do both kd's sigma
                # transposes + squares first so the squares run before the
                # ACT engine saturates with epilogue chunk passes
                mtps = []
                for kd in range(KD):
                    stp = tpp.tile([128, 512], f32, tag="tp")
                    for jj in range(nj):
                        j = j0 + jj
                        nc.tensor.transpose(
                            stp[:, ds(jj * 128, 128)],
                            sig_all[:, ds(j * D + kd * 128, 128)],
                            ident[:, :],
                        )
                    nc.scalar.square(w1t[kd][:, jgs], stp[:, 0 : nj * 128])
                for kd in range(KD):
                    mtp = tpp.tile([128, 512], f32, tag="tp")
                    for jj in range(nj):
                        j = j0 + jj
                        nc.tensor.transpose(
                            mtp[:, ds(jj * 128, 128)],
                            mu_all[:, ds(j * D + kd * 128, 128)],
                            ident[:, :],
                        )
                    nc.vector.tensor_mul(
                        w2t[kd][:, jgs], w1t[kd][:, jgs], mtp[:, 0 : nj * 128]
                    )
                    mtps.append(mtp)
                for kd in range(KD):
                    w3 = w3p.tile(
                        [128, 512], fr, tag="w3", bufs=6, name=f"w3_{gi}_{kd}"
                    )
                    nc.vector.tensor_mul(
                        w3[:, 0 : nj * 128], w2t[kd][:, jgs], mtps[kd][:, 0 : nj * 128]
                    )
                    w3c[(gi, kd)] = w3

            for gi, (j0, nj) in enumerate(GROUPS):
                jgs = ds(j0 * 128, nj * 128)
                for mi in range(MT):
                    ops = ops_mi[mi]
                    for kd in range(KD):
                        nc.tensor.matmul(
                            ops[:, jgs],
                            xsqT[kd][:, ds(mi * 128, 128)],
                            w1t[kd][:, jgs],
                            start=(kd == 0),
                            stop=False,
                        )
                    for kd in range(KD):
                        nc.tensor.matmul(
                            ops[:, jgs],
                            xm2T[kd][:, ds(mi * 128, 128)],
                            w2t[kd][:, jgs],
                            start=False,
                            stop=False,
                        )
                    for kd in range(KD):
                        nc.tensor.matmul(
                            ops[:, jgs],
                            ones_t[:, :],
                            w3c[(gi, kd)][:, 0 : nj * 128],
                            start=False,
                            stop=(kd == KD - 1),
                        )
                for mi in range(MT):
                    lg = epip.tile([128, 512], f32, tag="lg")
                    nc.scalar.activation(
                        lg[:, 0 : nj * 128], ops_mi[mi][:, jgs], AF.Ln
                    )
                    dist = epip.tile([128, 512], f32, tag="dist")
                    nc.scalar.activation(
                        dist[:, 0 : nj * 128], lg[:, 0 : nj * 128], AF.Exp, scale=0.5
                    )
                    nc.scalar.activation(
                        loc_mi[mi][:, jgs], dist[:, 0 : nj * 128], AF.Exp, scale=-1.0
                    )
                    if gi == len(GROUPS) - 1:
                        # this m-tile is complete: exp(g*loc) 1024-wide with
                        # fused row-sum, then normalize + store, emitted
                        # before the next m-tile's chunk passes so the
                        # normalize/DMA tail starts as early as possible
                        e_t = epip.tile([128, F], f32, tag="e")
                        s_col = smallp.tile([128, 1], f32, tag="ssum")
                        nc.scalar.activation(
                            e_t[:, :],
                            loc_mi[mi][:, :],
                            AF.Exp,
                            scale=g_col[:, 0:1],
                            accum_out=s_col[:, 0:1],
                        )
                        r_col = smallp.tile([128, 1], f32, tag="r")
                        nc.vector.reciprocal(r_col[:, :], s_col[:, :])
                        for jo in range(JG):
                            jos = ds(jo * 512, 512)
                            out_sb = epip.tile([128, 512], f32, tag="outsb", bufs=4)
                            nc.vector.tensor_scalar_mul(
                                out_sb[:, :], e_t[:, jos], r_col[:, 0:1]
                            )
                            nc.sync.dma_start(
                                out_d[ds(mi * 128, 128), jos], out_sb[:, :]
                            )

    nc.compile()
    return nc


LAST_RESULT = {}


def kernel(inputs, mu, sigma, temperature):
    inputs = np.ascontiguousarray(np.asarray(inputs, dtype=np.float32))
    mu = np.ascontiguousarray(np.asarray(mu, dtype=np.float32))
    sigma = np.ascontiguousarray(np.asarray(sigma, dtype=np.float32)).reshape(F, D)
    temp = np.asarray(temperature, dtype=np.float32).reshape(1, 1)

    from concourse.bass_utils import run_bass_kernel_spmd

    nc = build_bass()

    in_maps = []
    for i in range(NCORES):
        in_maps.append(
            {
                "x": inputs[i * BC : (i + 1) * BC],
                "mu": mu,
                "sigma": sigma,
                "temp": temp,
            }
        )

    trace = bool(int(os.environ.get("KERNEL_TRACE", "0")))
    res = run_bass_kernel_spmd(
        nc,
        in_maps,
        core_ids=list(range(NCORES)),
        trace=trace,
    )
    LAST_RESULT["exec_time_ns"] = res.exec_time_ns
    LAST_RESULT["mean_exec_time_ns"] = res.mean_exec_time_ns
    LAST_RESULT["trace"] = res.instructions_and_trace

    out = np.concatenate([res.results[i]["out"] for i in range(NCORES)], axis=0)
    return out
